# revision 2
# baseline (speedup 1.0000x reference)
"""B-spline (clamped, degree-3 by default) surface evaluation on 8 Trainium2 cores.

Math: out[u, v, :] = sum_{a,b} Bu[u,a] * Bv[v,b] * P[su[u]-p+a, sv[v]-p+b, :]

Host precomputes the tiny Cox-de-Boor basis and scatters it into dense
matrices Au [Nu, 64], Av [Nv, 64] so the device work is two dense matmul
stages (TensorEngine friendly, no gathers):

  stage 1:  Tt_d[j, u] = sum_i P[i, j, d] * Au[u, i]          (3 small matmuls)
  stage 2:  S[u, v, d] = sum_j Tt_d[j, u] * Av[v, j]          (tiled matmuls)

Sharding: data-parallel over u (rows of Au). Each of the 8 cores computes a
[251, 2001, 3] row-slab of the output; host concatenates and trims.
"""

import numpy as np

N_CTRL = 64
N_EVAL = 2001
N_CORES = 8
NU_SHARD = 251  # ceil(2001 / 8); 8 * 251 = 2008 (last 7 rows are zero padding)
U_TILES = [(0, 128), (128, NU_SHARD - 128)]
V_TILE = 512

_CACHE = {}


def _clamped_knots(p, n_ctrl, dtype=np.float64):
    n_internal = n_ctrl - p - 1
    internal = np.linspace(0.0, 1.0, n_internal + 2, dtype=dtype)[1:-1]
    return np.concatenate(
        [np.zeros(p + 1, dtype), internal, np.ones(p + 1, dtype)]
    )


def _dense_basis(params, p, n_ctrl):
    """Dense basis matrix A [len(params), n_ctrl] with A[k, span-p+a] = B[k, a]."""
    knots = _clamped_knots(p, n_ctrl)
    u = np.asarray(params, np.float64)
    spans = np.clip(np.searchsorted(knots, u, side="right") - 1, p, n_ctrl - 1)
    Ns = [np.ones_like(u)]
    left = {}
    right = {}
    for j in range(1, p + 1):
        left[j] = u - knots[spans + 1 - j]
        right[j] = knots[spans + j] - u
        saved = np.zeros_like(u)
        new = []
        for r in range(j):
            temp = Ns[r] / (right[r + 1] + left[j - r])
            new.append(saved + right[r + 1] * temp)
            saved = left[j - r] * temp
        new.append(saved)
        Ns = new
    B = np.stack(Ns, axis=-1)  # [N, p+1]
    A = np.zeros((len(u), n_ctrl), np.float32)
    rows = np.arange(len(u))[:, None]
    cols = spans[:, None] - p + np.arange(p + 1)[None, :]
    A[rows, cols] = B.astype(np.float32)
    return A


def _build_device():
    if "nc" in _CACHE:
        return _CACHE["nc"]

    import concourse.mybir as mybir
    import concourse.tile as tile
    from concourse import bacc

    f32 = mybir.dt.float32
    nc = bacc.Bacc(
        "TRN2", target_bir_lowering=False, debug=False, num_devices=N_CORES
    )
    aut_h = nc.dram_tensor("aut", [N_CTRL, NU_SHARD], f32, kind="ExternalInput").ap()
    avt_h = nc.dram_tensor("avt", [N_CTRL, N_EVAL], f32, kind="ExternalInput").ap()
    pp_h = nc.dram_tensor("pperm", [N_CTRL, 3 * N_CTRL], f32, kind="ExternalInput").ap()
    out_h = nc.dram_tensor(
        "out", [NU_SHARD, 3 * N_EVAL], f32, kind="ExternalOutput"
    ).ap()

    with tile.TileContext(nc) as tc:
        with (
            tc.tile_pool(name="consts", bufs=1) as consts,
            tc.tile_pool(name="ps1", bufs=2, space="PSUM") as ps1,
            tc.tile_pool(name="ps2", bufs=4, space="PSUM") as ps2,
            tc.tile_pool(name="obuf", bufs=4) as obuf,
        ):
            avt_sb = consts.tile([N_CTRL, N_EVAL], f32)
            nc.sync.dma_start(out=avt_sb, in_=avt_h)
            pp_sb = consts.tile([N_CTRL, 3 * N_CTRL], f32)
            nc.sync.dma_start(out=pp_sb, in_=pp_h)
            aut_sb = consts.tile([N_CTRL, NU_SHARD], f32)
            nc.sync.dma_start(out=aut_sb, in_=aut_h)

            # stage 1: Tt_d[j, u] = sum_i pperm[i, d*64+j] * aut[i, u]
            tt_sb = consts.tile([N_CTRL, 3 * NU_SHARD], f32)
            for d in range(3):
                pt = ps1.tile([N_CTRL, NU_SHARD], f32, tag="pt")
                nc.tensor.matmul(
                    pt,
                    pp_sb[:, d * N_CTRL : (d + 1) * N_CTRL],
                    aut_sb,
                    start=True,
                    stop=True,
                )
                nc.vector.tensor_copy(
                    tt_sb[:, d * NU_SHARD : (d + 1) * NU_SHARD], pt
                )

            # stage 2: S[u, v, d] = sum_j Tt_d[j, u] * Av[v, j], interleaved to
            # row layout [u, v*3 + d] and DMA'd out per (u-tile, v-tile) chunk.
            for u0, uw in U_TILES:
                for iv, v0 in enumerate(range(0, N_EVAL, V_TILE)):
                    vw = min(V_TILE, N_EVAL - v0)
                    ob = obuf.tile([128, 3 * V_TILE], f32, tag="ob")
                    for d in range(3):
                        ps = ps2.tile([128, V_TILE], f32, tag="ps")
                        nc.tensor.matmul(
                            ps[:uw, :vw],
                            tt_sb[:, d * NU_SHARD + u0 : d * NU_SHARD + u0 + uw],
                            avt_sb[:, v0 : v0 + vw],
                            start=True,
                            stop=True,
                        )
                        dst = ob[:uw, d : 3 * vw : 3]
                        # split the interleave copies across DVE and ACT
                        if d == 0 or (d == 2 and iv % 2 == 0):
                            nc.vector.tensor_copy(dst, ps[:uw, :vw])
                        else:
                            nc.scalar.copy(dst, ps[:uw, :vw])
                    nc.sync.dma_start(
                        out=out_h[u0 : u0 + uw, 3 * v0 : 3 * v0 + 3 * vw],
                        in_=ob[:uw, : 3 * vw],
                    )
    nc.compile()
    _CACHE["nc"] = nc
    return nc


def kernel(control_points, params_u, params_v, degree):
    from concourse.bass_utils import run_bass_kernel_spmd

    p = int(np.asarray(degree))
    cp = np.asarray(control_points, np.float32)
    pu = np.asarray(params_u, np.float32)
    pv = np.asarray(params_v, np.float32)
    assert cp.shape == (N_CTRL, N_CTRL, 3), cp.shape
    assert pu.shape == (N_EVAL,) and pv.shape == (N_EVAL,), (pu.shape, pv.shape)

    Au = np.zeros((N_CORES * NU_SHARD, N_CTRL), np.float32)
    Au[:N_EVAL] = _dense_basis(pu, p, N_CTRL)
    Av = _dense_basis(pv, p, N_CTRL)

    avt = np.ascontiguousarray(Av.T)
    pperm = np.ascontiguousarray(cp.transpose(0, 2, 1).reshape(N_CTRL, 3 * N_CTRL))

    nc = _build_device()
    in_maps = []
    for c in range(N_CORES):
        aut_c = np.ascontiguousarray(Au[c * NU_SHARD : (c + 1) * NU_SHARD].T)
        in_maps.append({"aut": aut_c, "avt": avt, "pperm": pperm})

    res = run_bass_kernel_spmd(
        nc,
        in_maps,
        core_ids=list(range(N_CORES)),
        trace=_CACHE.get("trace", False),
        **_CACHE.get("run_kwargs", {}),
    )
    _CACHE["last_result"] = res
    full = np.concatenate([r["out"] for r in res.results], axis=0)[:N_EVAL]
    return np.ascontiguousarray(full.reshape(N_EVAL, N_EVAL, 3))


# revision 4
# speedup vs baseline: 1.6653x; 1.6653x over previous
"""B-spline (clamped) surface evaluation on 8 Trainium2 cores.

Math: out[u, v, :] = sum_{a,b} Bu[u,a] * Bv[v,b] * P[su[u]-p+a, sv[v]-p+b, :]

Host precomputes the tiny Cox-de-Boor basis and scatters it into dense
matrices Au [Nu, 64], Av [Nv, 64] so the device work is two dense matmul
stages (TensorEngine friendly, no gathers):

  stage 1:  Tt_d[j, u] = sum_i P[i, j, d] * Au[u, i]          (small matmuls)
  stage 2:  S[u, v, d] = sum_j Tt_d[j, u] * Av[v, j]          (tiled matmuls)

fp32 matmul on TRN2 runs ~5x slower than bf16 (LOW_HIGH 2-pass weights x
2 cyc/col fp32 streaming), so all matmuls use a 3-pass bf16 hi/lo split
(a = hi + lo, drop the lo*lo term; ~7e-6 relative error, fp32 PSUM accum).

Sharding: data-parallel over u (rows of Au). Each of the 8 cores computes a
[251, 2001, 3] row-slab of the output; host concatenates and trims.
"""

import numpy as np

N_CTRL = 64
N_EVAL = 2001
N_CORES = 8
NU_SHARD = 251  # ceil(2001 / 8); 8 * 251 = 2008 (last 7 rows are zero padding)
U_TILES = [(0, 128), (128, NU_SHARD - 128)]
V_TILE = 512

_CACHE = {}


def _clamped_knots(p, n_ctrl, dtype=np.float64):
    n_internal = n_ctrl - p - 1
    internal = np.linspace(0.0, 1.0, n_internal + 2, dtype=dtype)[1:-1]
    return np.concatenate(
        [np.zeros(p + 1, dtype), internal, np.ones(p + 1, dtype)]
    )


def _dense_basis(params, p, n_ctrl):
    """Dense basis matrix A [len(params), n_ctrl] with A[k, span-p+a] = B[k, a]."""
    knots = _clamped_knots(p, n_ctrl)
    u = np.asarray(params, np.float64)
    spans = np.clip(np.searchsorted(knots, u, side="right") - 1, p, n_ctrl - 1)
    Ns = [np.ones_like(u)]
    left = {}
    right = {}
    for j in range(1, p + 1):
        left[j] = u - knots[spans + 1 - j]
        right[j] = knots[spans + j] - u
        saved = np.zeros_like(u)
        new = []
        for r in range(j):
            temp = Ns[r] / (right[r + 1] + left[j - r])
            new.append(saved + right[r + 1] * temp)
            saved = left[j - r] * temp
        new.append(saved)
        Ns = new
    B = np.stack(Ns, axis=-1)  # [N, p+1]
    A = np.zeros((len(u), n_ctrl), np.float32)
    rows = np.arange(len(u))[:, None]
    cols = spans[:, None] - p + np.arange(p + 1)[None, :]
    A[rows, cols] = B.astype(np.float32)
    return A


def _split_bf16(a):
    """fp32 array -> (hi, lo) bf16 arrays with hi + lo ~= a (~2^-18 rel)."""
    import ml_dtypes

    a = np.ascontiguousarray(a, np.float32)
    hi = a.astype(ml_dtypes.bfloat16)
    lo = (a - hi.astype(np.float32)).astype(ml_dtypes.bfloat16)
    return np.ascontiguousarray(hi), np.ascontiguousarray(lo)


def _build_device():
    if "nc" in _CACHE:
        return _CACHE["nc"]

    import concourse.mybir as mybir
    import concourse.tile as tile
    from concourse import bacc

    f32 = mybir.dt.float32
    bf16 = mybir.dt.bfloat16
    nc = bacc.Bacc(
        "TRN2", target_bir_lowering=False, debug=False, num_devices=N_CORES
    )
    ins = {}
    for name, cols in [("aut", NU_SHARD), ("pperm", 3 * N_CTRL), ("avt", N_EVAL)]:
        for part in ("hi", "lo"):
            ins[f"{name}_{part}"] = nc.dram_tensor(
                f"{name}_{part}", [N_CTRL, cols], bf16, kind="ExternalInput"
            ).ap()
    out_h = nc.dram_tensor(
        "out", [NU_SHARD, 3 * N_EVAL], f32, kind="ExternalOutput"
    ).ap()

    with tile.TileContext(nc) as tc:
        with (
            tc.tile_pool(name="consts", bufs=1) as consts,
            tc.tile_pool(name="ps1", bufs=2, space="PSUM") as ps1,
            tc.tile_pool(name="ps2", bufs=4, space="PSUM") as ps2,
            tc.tile_pool(name="obuf", bufs=4) as obuf,
        ):
            sb = {}
            # stage-1 inputs first so stage 1 can start while avt streams in
            for name, cols in [("aut", NU_SHARD), ("pperm", 3 * N_CTRL), ("avt", N_EVAL)]:
                for part in ("hi", "lo"):
                    key = f"{name}_{part}"
                    sb[key] = consts.tile([N_CTRL, cols], bf16, tag=key, name=key)
                    nc.sync.dma_start(out=sb[key], in_=ins[key])

            # stage 1: Tt_d[j, u] = sum_i pperm[i, d*64+j] * aut[i, u]  (3-pass)
            tt_sb = consts.tile([N_CTRL, 3 * NU_SHARD], f32)
            for d in range(3):
                pt = ps1.tile([N_CTRL, NU_SHARD], f32, tag="pt")
                dsl = slice(d * N_CTRL, (d + 1) * N_CTRL)
                nc.tensor.matmul(
                    pt, sb["pperm_hi"][:, dsl], sb["aut_hi"], start=True, stop=False
                )
                nc.tensor.matmul(
                    pt, sb["pperm_hi"][:, dsl], sb["aut_lo"], start=False, stop=False
                )
                nc.tensor.matmul(
                    pt, sb["pperm_lo"][:, dsl], sb["aut_hi"], start=False, stop=True
                )
                nc.vector.tensor_copy(
                    tt_sb[:, d * NU_SHARD : (d + 1) * NU_SHARD], pt
                )

            # split Tt (fp32) into bf16 hi/lo for stage 2
            tt_hi = consts.tile([N_CTRL, 3 * NU_SHARD], bf16)
            tt_hi32 = consts.tile([N_CTRL, 3 * NU_SHARD], f32)
            tt_lo32 = consts.tile([N_CTRL, 3 * NU_SHARD], f32)
            tt_lo = consts.tile([N_CTRL, 3 * NU_SHARD], bf16)
            nc.vector.tensor_copy(tt_hi, tt_sb)
            nc.vector.tensor_copy(tt_hi32, tt_hi)
            nc.vector.tensor_sub(tt_lo32, tt_sb, tt_hi32)
            nc.vector.tensor_copy(tt_lo, tt_lo32)

            # stage 2: S[u, v, d] = sum_j Tt_d[j, u] * Av[v, j]  (3-pass),
            # interleaved to row layout [u, v*3 + d], DMA'd per (u-tile, v-tile).
            for u0, uw in U_TILES:
                for iv, v0 in enumerate(range(0, N_EVAL, V_TILE)):
                    vw = min(V_TILE, N_EVAL - v0)
                    vsl = slice(v0, v0 + vw)
                    ob = obuf.tile([128, 3 * V_TILE], f32, tag="ob")
                    for d in range(3):
                        usl = slice(d * NU_SHARD + u0, d * NU_SHARD + u0 + uw)
                        ps = ps2.tile([128, V_TILE], f32, tag="ps")
                        nc.tensor.matmul(
                            ps[:uw, :vw], tt_hi[:, usl], sb["avt_hi"][:, vsl],
                            start=True, stop=False,
                        )
                        nc.tensor.matmul(
                            ps[:uw, :vw], tt_hi[:, usl], sb["avt_lo"][:, vsl],
                            start=False, stop=False,
                        )
                        nc.tensor.matmul(
                            ps[:uw, :vw], tt_lo[:, usl], sb["avt_hi"][:, vsl],
                            start=False, stop=True,
                        )
                        dst = ob[:uw, d : 3 * vw : 3]
                        # split the interleave copies across DVE and ACT
                        if d == 0 or (d == 2 and iv % 2 == 0):
                            nc.vector.tensor_copy(dst, ps[:uw, :vw])
                        else:
                            nc.scalar.copy(dst, ps[:uw, :vw])
                    # SWDGE (gpsimd) spreads descriptors over all 16 SDMA
                    # engines; the HWDGE path was observed pinning most of the
                    # output bytes on 3 engines (~80 GB/s).
                    nc.gpsimd.dma_start(
                        out=out_h[u0 : u0 + uw, 3 * v0 : 3 * v0 + 3 * vw],
                        in_=ob[:uw, : 3 * vw],
                    )
    nc.compile()
    _CACHE["nc"] = nc
    return nc


def kernel(control_points, params_u, params_v, degree):
    from concourse.bass_utils import run_bass_kernel_spmd

    p = int(np.asarray(degree))
    cp = np.asarray(control_points, np.float32)
    pu = np.asarray(params_u, np.float32)
    pv = np.asarray(params_v, np.float32)
    assert cp.shape == (N_CTRL, N_CTRL, 3), cp.shape
    assert pu.shape == (N_EVAL,) and pv.shape == (N_EVAL,), (pu.shape, pv.shape)

    Au = np.zeros((N_CORES * NU_SHARD, N_CTRL), np.float32)
    Au[:N_EVAL] = _dense_basis(pu, p, N_CTRL)
    Av = _dense_basis(pv, p, N_CTRL)

    avt_hi, avt_lo = _split_bf16(Av.T)
    pperm = cp.transpose(0, 2, 1).reshape(N_CTRL, 3 * N_CTRL)
    pperm_hi, pperm_lo = _split_bf16(pperm)

    nc = _build_device()
    in_maps = []
    for c in range(N_CORES):
        aut_hi, aut_lo = _split_bf16(Au[c * NU_SHARD : (c + 1) * NU_SHARD].T)
        in_maps.append(
            {
                "aut_hi": aut_hi,
                "aut_lo": aut_lo,
                "avt_hi": avt_hi,
                "avt_lo": avt_lo,
                "pperm_hi": pperm_hi,
                "pperm_lo": pperm_lo,
            }
        )

    res = run_bass_kernel_spmd(
        nc,
        in_maps,
        core_ids=list(range(N_CORES)),
        trace=_CACHE.get("trace", False),
        **_CACHE.get("run_kwargs", {}),
    )
    _CACHE["last_result"] = res
    full = np.concatenate([r["out"] for r in res.results], axis=0)[:N_EVAL]
    return np.ascontiguousarray(full.reshape(N_EVAL, N_EVAL, 3))
